# revision 1
# baseline (speedup 1.0000x reference)
"""Trainium2 Bass kernel for nn_CCNN2d (convolutional coupled NN 2d).

Per-sample recurrence (T steps), per core (pure data parallel over B=8):
    f = DF*f + conv3x3(y, conv_w) + bias + x_t
    l = DL*l + conv3x3(y, wf)          # wf[o,i,:,:] = k  (same for all o,i)
    u = f * (1 + 0.5*l)
    e = DE*e + VE*y      (tracked as ehat = e/VE;  ehat = DE*ehat + y)
    y = sigmoid(u - e)   (= sigmoid(u - VE*ehat))

Layout (per core): C=32 channels, H=128 rows in G=4 bands of 32 rows.
SBUF partition 32*g + c  <-  (band g, channel c).
Conv as matmul on 32x32 PE subarray tiles:
  cf tile (row g, col g): 9 conv taps + identity*x tap + ones*bias/32 tap.
  lc tile (row g, col (g+1)%4): 8 taps of k[dy,dx]*ones (center k=0).
    -> L state kept in "rotated" layout (group g+1 holds band g).
  T1 = 1 + 0.5*L (rotated) is rotated back to natural via identity matmuls
    through the PE into PSUM, then u = F * T1psum on DVE.
y kept in a padded plane per band: (BR+2) x (W+2) with halo rows copied
from neighbour bands (32-partition cross-quadrant DVE copies) each step.
"""

import numpy as np

import concourse.bass as bass
import concourse.mybir as mybir
import concourse.tile as tile
from concourse.bass_utils import run_bass_kernel_spmd


F32 = mybir.dt.float32
F32R = mybir.dt.float32r
BF16 = mybir.dt.bfloat16
ALU = mybir.AluOpType
ACTF = mybir.ActivationFunctionType

ALPHA_F, ALPHA_L, ALPHA_E, V_E = 0.1, 1.0, 1.0, 10.0
DF = float(np.exp(-ALPHA_F))
DL = float(np.exp(-ALPHA_L))
DE = float(np.exp(-ALPHA_E))

B, T, C, H, W = 8, 10, 32, 128, 128
G = 4  # partition groups / row bands
KVALS = [[0.5, 1.0, 0.5], [1.0, 0.0, 1.0], [0.5, 1.0, 0.5]]


def build_nc(t_steps=T, h=H, w=W):
    br = h // G          # rows per band
    pw = w + 2           # padded width
    ph = br + 2          # padded rows per band
    n_chunks = br // 8   # 8 image rows (2 psum banks) per chunk
    assert br % 8 == 0 and w == 128

    nc = bass.Bass()
    x_ext = nc.declare_dram_parameter("x", [t_steps, C, h, w], F32, isOutput=False)
    w_ext = nc.declare_dram_parameter("conv_w", [C, C, 3, 3], F32, isOutput=False)
    b_ext = nc.declare_dram_parameter("conv_b", [C], F32, isOutput=False)
    y_ext = nc.declare_dram_parameter("y", [t_steps, C, h, w], F32, isOutput=True)

    from contextlib import ExitStack
    with tile.TileContext(nc) as tc, ExitStack() as ctx:
        _build(ctx, tc, nc, x_ext, w_ext, b_ext, y_ext, t_steps, h, w, br, ph, pw, n_chunks)
    return nc


def _split_matmul_waits(nc):
    """walrus's S3_LW (matmul weight-load) struct has a single sync-wait
    slot; Tile sometimes attaches 2+. Move all-but-one wait onto an
    inserted PE EventSemaphore right before the matmul."""
    import copy as _copy

    split_types = {
        "InstMatmult", "InstTensorTensor", "InstTensorScalarPtr",
        "InstActivation", "InstTensorCopy", "InstStreamTranspose",
        "InstMemset", "InstTensorScalarAffineSelect", "InstTensorReduce",
        "InstDMACopy", "InstTensorLoad", "InstTensorSave", "InstDrain", "InstNoOp",
    }
    fn = nc.m.functions[0]
    new_blocks = []
    for bb in fn.blocks:
        out = []
        changed = False
        for ins in bb.instructions:
            si = ins.sync_info
            if (type(ins).__name__ in split_types and si is not None
                    and si.on_wait and len(si.on_wait) > 1):
                waits = list(si.on_wait)
                for i in range(0, len(waits), 2):  # <=2 waits per EventSemaphore
                    out.append(mybir.InstEventSemaphore(
                        name=nc.get_next_instruction_name(),
                        engine=ins.engine,
                        ins=[], outs=[],
                        sync_info=mybir.SyncInfo(
                            on_wait=waits[i:i + 2], on_update=[]),
                    ))
                ins.sync_info = mybir.SyncInfo(
                    on_wait=[], on_update=list(si.on_update or []))
                changed = True
            out.append(ins)
        if changed:
            new_blocks.append(_copy.replace(bb, instructions=out))
        else:
            new_blocks.append(bb)
    new_fn = _copy.replace(fn, blocks=[])
    new_fn.set_allocations_from_list(fn.allocations)
    new_fn.blocks.extend(new_blocks)
    nc.m = _copy.replace(nc.m, functions=[])
    nc.m.functions.append(new_fn)


def _build(ctx, tc, nc, x_ext, w_ext, b_ext, y_ext, t_steps, h, w, br, ph, pw, n_chunks):
    singles = ctx.enter_context(tc.tile_pool(name="singles", bufs=1))
    xbpool = ctx.enter_context(tc.tile_pool(name="xbpool", bufs=1))
    xpool = ctx.enter_context(tc.tile_pool(name="xpool", bufs=2))
    tmp_pool = ctx.enter_context(tc.tile_pool(name="tmps", bufs=1))
    psA_pool = ctx.enter_context(tc.tile_pool(name="psA", bufs=2, space="PSUM"))
    psB_pool = ctx.enter_context(tc.tile_pool(name="psB", bufs=2, space="PSUM"))

    # ---- persistent state ----
    Yf0 = singles.tile([128, br * w], F32, tag="Yf0")   # flat fp32 y (ping)
    Yf1 = singles.tile([128, br * w], F32, tag="Yf1")   # flat fp32 y (pong)
    Yh0 = singles.tile([128, ph * pw], BF16, tag="Yh0")  # padded bf16 hi
    Yl0 = singles.tile([128, ph * pw], BF16, tag="Yl0")  # padded bf16 lo
    Yh1 = singles.tile([128, ph * pw], BF16, tag="Yh1")
    Yl1 = singles.tile([128, ph * pw], BF16, tag="Yl1")
    F = singles.tile([128, br * w], F32, tag="F")
    L = singles.tile([128, br * w], F32, tag="L")   # rotated: group (g+1)%4 = band g
    E = singles.tile([128, br * w], F32, tag="E")   # ehat = e / V_E

    # ---- constants ----
    Wcf = singles.tile([128, 9 * 32], F32, tag="Wcf")    # fp32 staging, replicated
    Wch = singles.tile([128, 9 * 32], BF16, tag="Wch")   # bf16 hi
    Wcl = singles.tile([128, 9 * 32], BF16, tag="Wcl")   # bf16 lo
    Wstage = singles.tile([32, 9 * 32], F32, tag="Wstage")
    Wbf = singles.tile([128, 32], F32, tag="Wbf")        # bias/32 fp32
    Wbh = singles.tile([128, 32], BF16, tag="Wbh")
    Wbl = singles.tile([128, 32], BF16, tag="Wbl")
    IDT = singles.tile([128, 32], BF16, tag="IDT")
    K05 = singles.tile([128, 32], BF16, tag="K05")
    K10 = singles.tile([128, 32], BF16, tag="K10")
    ONES = singles.tile([128, 512], BF16, tag="ONES")

    for yy in (Yh0, Yl0, Yh1, Yl1):
        nc.vector.memset(yy, 0.0)
    nc.vector.memset(Yf0, 0.0)
    nc.vector.memset(Yf1, 0.0)
    nc.vector.memset(F, 0.0)
    nc.vector.memset(L, 0.0)
    # e0 = V_E/ALPHA_E, ehat0 = e0/V_E = 1/ALPHA_E
    nc.vector.memset(E, 1.0 / ALPHA_E)
    nc.vector.memset(K05, 0.5)
    nc.vector.memset(K10, 1.0)
    nc.vector.memset(ONES, 1.0)

    # conv weights: stage as [o, (i ky kx)] contiguous, transpose per tap to [i, o]
    nc.sync.dma_start(out=Wstage, in_=w_ext[:].rearrange("o i ky kx -> o (i ky kx)"))
    wst = Wstage.rearrange("p (i t) -> p i t", t=9)
    for tap in range(9):
        nc.vector.transpose(out=Wcf[0:32, 32 * tap:32 * tap + 32], in_=wst[:, :, tap])
    # identity: gpsimd can't touch f32r, so build in an f32 staging tile
    # and cast-copy into IDT on DVE (exact for 0/1 values)
    IDTs = singles.tile([32, 32], F32, tag="IDTs")
    nc.gpsimd.memset(IDTs, 0.0)
    nc.gpsimd.affine_select(
        out=IDTs, in_=IDTs,
        compare_op=ALU.not_equal, fill=1.0, base=0,
        pattern=[[-1, 32]], channel_multiplier=1)
    nc.vector.tensor_copy(out=IDT[0:32, :], in_=IDTs)
    # bias/32, partition-broadcast to all 128 partitions
    b_ap = b_ext[:]
    b_bcast = bass.AP(tensor=b_ap.tensor, offset=b_ap.offset, ap=[[0, 128]] + list(b_ap.ap))
    nc.sync.dma_start(out=Wbf, in_=b_bcast)
    nc.vector.tensor_scalar_mul(Wbf, Wbf, 1.0 / 32.0)
    # replicate fp32 weights to groups 1..3, then hi/lo bf16 split
    for g in range(1, G):
        nc.vector.tensor_copy(out=Wcf[32 * g:32 * g + 32, :], in_=Wcf[0:32, :])
        nc.vector.tensor_copy(out=IDT[32 * g:32 * g + 32, :], in_=IDT[0:32, :])
    nc.vector.tensor_copy(out=Wch, in_=Wcf)
    nc.vector.scalar_tensor_tensor(out=Wcl, in0=Wcf, scalar=1.0, in1=Wch,
                                   op0=ALU.mult, op1=ALU.subtract)
    nc.vector.tensor_copy(out=Wbh, in_=Wbf)
    nc.vector.scalar_tensor_tensor(out=Wbl, in0=Wbf, scalar=1.0, in1=Wbh,
                                   op0=ALU.mult, op1=ALU.subtract)

    taps = [(ky, kx) for ky in range(3) for kx in range(3)]
    lc_taps = [(ky, kx) for (ky, kx) in taps if KVALS[ky][kx] != 0.0]

    for t in range(t_steps):
        yf_in = Yf0 if t % 2 == 0 else Yf1
        yf_out = Yf1 if t % 2 == 0 else Yf0
        yh_in, yl_in = (Yh0, Yl0) if t % 2 == 0 else (Yh1, Yl1)
        yh_out, yl_out = (Yh1, Yl1) if t % 2 == 0 else (Yh0, Yl0)
        yhv = yh_in.rearrange("p (r c) -> p r c", c=pw)
        ylv = yl_in.rearrange("p (r c) -> p r c", c=pw)
        yhov = yh_out.rearrange("p (r c) -> p r c", c=pw)
        ylov = yl_out.rearrange("p (r c) -> p r c", c=pw)

        xb = xbpool.tile([128, br * w], F32, tag="xb")
        nc.sync.dma_start(out=xb, in_=x_ext[t].rearrange("c (g r) w -> g c r w", g=G))
        xh = xpool.tile([128, br * w], BF16, tag="xh")
        xl = xpool.tile([128, br * w], BF16, tag="xl")
        nc.vector.tensor_copy(out=xh, in_=xb)
        nc.vector.scalar_tensor_tensor(out=xl, in0=xb, scalar=1.0, in1=xh,
                                       op0=ALU.mult, op1=ALU.subtract)

        for ch in range(n_chunks):
            sl = slice(1024 * ch, 1024 * ch + 1024)
            pA = psA_pool.tile([128, 1024], F32, tag="pA")
            pB = psB_pool.tile([128, 1024], F32, tag="pBT")
            for lb in range(2):            # local bank
                gb = 2 * ch + lb           # global 4-row block index
                co = 512 * lb
                # cf: 27 split-conv taps + x hi/lo + bias hi/lo (diagonal tiles)
                for ti, (ky, kx) in enumerate(taps):
                    for (wt, yv) in ((Wch, yhv), (Wch, ylv), (Wcl, yhv)):
                        for g in range(G):
                            nc.tensor.matmul(
                                out=pA[32 * g:32 * g + 32, co:co + 512],
                                lhsT=wt[32 * g:32 * g + 32, 32 * ti:32 * ti + 32],
                                rhs=yv[32 * g:32 * g + 32, 4 * gb + ky:4 * gb + ky + 4,
                                       kx:kx + 128],
                                start=(ti == 0 and wt is Wch and yv is yhv),
                                stop=False, skip_group_check=True,
                                tile_position=(32 * g, 32 * g),
                            )
                for xt in (xh, xl):
                    for g in range(G):
                        nc.tensor.matmul(
                            out=pA[32 * g:32 * g + 32, co:co + 512],
                            lhsT=IDT[32 * g:32 * g + 32, 0:32],
                            rhs=xt[32 * g:32 * g + 32, 512 * gb:512 * gb + 512],
                            start=False, stop=False, skip_group_check=True,
                            tile_position=(32 * g, 32 * g),
                        )
                for bi, bt in enumerate((Wbh, Wbl)):
                    for g in range(G):
                        nc.tensor.matmul(
                            out=pA[32 * g:32 * g + 32, co:co + 512],
                            lhsT=bt[32 * g:32 * g + 32, 0:32],
                            rhs=ONES[32 * g:32 * g + 32, 0:512],
                            start=False, stop=(bi == 1), skip_group_check=True,
                            tile_position=(32 * g, 32 * g),
                        )
                # lc: 8 taps x (yh, yl), output rotated by one group
                nlc = len(lc_taps)
                for ti, (ky, kx) in enumerate(lc_taps):
                    kt = K05 if KVALS[ky][kx] == 0.5 else K10
                    for yi, yv in enumerate((yhv, ylv)):
                        for g in range(G):
                            g2 = (g + 1) % G
                            nc.tensor.matmul(
                                out=pB[32 * g2:32 * g2 + 32, co:co + 512],
                                lhsT=kt[32 * g:32 * g + 32, 0:32],
                                rhs=yv[32 * g:32 * g + 32, 4 * gb + ky:4 * gb + ky + 4,
                                       kx:kx + 128],
                                start=(ti == 0 and yi == 0),
                                stop=(ti == nlc - 1 and yi == 1),
                                skip_group_check=True, tile_position=(32 * g, 32 * g2),
                            )

            # elementwise updates for this chunk (8 image rows / band)
            nc.vector.scalar_tensor_tensor(
                out=F[:, sl], in0=F[:, sl], scalar=DF, in1=pA[:, :],
                op0=ALU.mult, op1=ALU.add)
            nc.vector.scalar_tensor_tensor(
                out=L[:, sl], in0=L[:, sl], scalar=DL, in1=pB[:, :],
                op0=ALU.mult, op1=ALU.add)
            T1f = tmp_pool.tile([128, 1024], F32, tag="T1f")
            T1h = tmp_pool.tile([128, 1024], BF16, tag="T1h")
            T1l = tmp_pool.tile([128, 1024], BF16, tag="T1l")
            nc.vector.tensor_scalar(T1f, L[:, sl], 0.5, 1.0, ALU.mult, ALU.add)
            nc.scalar.copy(out=T1h, in_=T1f)
            nc.vector.scalar_tensor_tensor(out=T1l, in0=T1f, scalar=1.0, in1=T1h,
                                           op0=ALU.mult, op1=ALU.subtract)
            pT = psB_pool.tile([128, 1024], F32, tag="pBT")
            for lb in range(2):
                co = 512 * lb
                for hi, tt in enumerate((T1h, T1l)):
                    for g in range(G):
                        g2 = (g + 1) % G
                        nc.tensor.matmul(
                            out=pT[32 * g:32 * g + 32, co:co + 512],
                            lhsT=IDT[32 * g2:32 * g2 + 32, 0:32],
                            rhs=tt[32 * g2:32 * g2 + 32, co:co + 512],
                            start=(hi == 0), stop=(hi == 1),
                            skip_group_check=True, tile_position=(32 * g2, 32 * g),
                        )
            U = tmp_pool.tile([128, 1024], F32, tag="U")
            nc.vector.tensor_tensor(out=U, in0=F[:, sl], in1=pT[:, :], op=ALU.mult)
            # ehat = DE*ehat + y_old (flat fp32 y)
            nc.vector.scalar_tensor_tensor(
                out=E[:, sl], in0=E[:, sl], scalar=DE, in1=yf_in[:, sl],
                op0=ALU.mult, op1=ALU.add)
            T2 = tmp_pool.tile([128, 1024], F32, tag="T2")
            nc.vector.scalar_tensor_tensor(
                out=T2, in0=E[:, sl], scalar=-V_E, in1=U,
                op0=ALU.mult, op1=ALU.add)
            nc.scalar.activation(out=yf_out[:, sl], in_=T2, func=ACTF.Sigmoid)
            # bf16 hi/lo split of new y into padded planes
            yho_int = yhov[:, 8 * ch + 1:8 * ch + 9, 1:1 + w]
            ylo_int = ylov[:, 8 * ch + 1:8 * ch + 9, 1:1 + w]
            nc.scalar.copy(out=yho_int, in_=yf_out[:, sl].rearrange("p (r c) -> p r c", c=w))
            nc.vector.scalar_tensor_tensor(
                out=ylo_int, in0=yf_out[:, sl].rearrange("p (r c) -> p r c", c=w),
                scalar=1.0, in1=yho_int, op0=ALU.mult, op1=ALU.subtract)
            y_dst = y_ext[t].rearrange("c (g r) w -> g c r w", g=G)
            for g in range(G):
                nc.sync.dma_start(
                    out=y_dst[g, :, 8 * ch:8 * ch + 8, :],
                    in_=yf_out[32 * g:32 * g + 32, sl])

        # halo rows for next step's convs
        if t + 1 < t_steps:
            for yv in (yhov, ylov):
                for g in range(1, G):
                    nc.vector.tensor_copy(out=yv[32 * g:32 * g + 32, 0, :],
                                          in_=yv[32 * (g - 1):32 * g, br, :])
                for g in range(G - 1):
                    nc.vector.tensor_copy(out=yv[32 * g:32 * g + 32, br + 1, :],
                                          in_=yv[32 * (g + 1):32 * (g + 2), 1, :])


_NC_CACHE = {}


def _get_nc(t_steps, h, w):
    key = (t_steps, h, w)
    if key not in _NC_CACHE:
        nc = build_nc(t_steps, h, w)
        _split_matmul_waits(nc)   # HW compile path only; CoreSim can't run these
        _NC_CACHE[key] = nc
    return _NC_CACHE[key]


def kernel(x, conv_w, conv_b):
    x = np.ascontiguousarray(np.asarray(x), dtype=np.float32)
    conv_w = np.ascontiguousarray(np.asarray(conv_w), dtype=np.float32)
    conv_b = np.ascontiguousarray(np.asarray(conv_b), dtype=np.float32)
    b, t_steps, c, h, w = x.shape
    nc = _get_nc(t_steps, h, w)
    in_maps = [
        {"x": x[i], "conv_w": conv_w, "conv_b": conv_b} for i in range(b)
    ]
    res = run_bass_kernel_spmd(nc, in_maps, core_ids=list(range(b)))
    return np.stack([res.results[i]["y"] for i in range(b)], axis=0)


if __name__ == "__main__":
    nc = build_nc()
    print("built", len(nc.m.functions[0].instructions) if hasattr(nc.m.functions[0], "instructions") else "ok")



# revision 11
# speedup vs baseline: 1.2249x; 1.2249x over previous
"""Trainium2 Bass kernel for nn_CCNN2d (convolutional coupled NN 2d).

Per-sample recurrence (T steps), per core (pure data parallel over B=8):
    f = DF*f + conv3x3(y, conv_w) + bias + x_t
    l = DL*l + conv3x3(y, wf)          # wf[o,i,:,:] = k  (same for all o,i)
    u = f * (1 + 0.5*l)
    e = DE*e + VE*y      (tracked as ehat = e/VE;  ehat = DE*ehat + y)
    y = sigmoid(u - e)   (= sigmoid(u - VE*ehat))

Layout (per core): C=32 channels, H=128 rows in G=4 bands of 32 rows.
SBUF partition 32*g + c  <-  (band g, channel c).
Conv as matmul on 32x32 PE subarray tiles:
  cf chain (row g, col g+rot): 27 split-conv taps + identity*(x+bias) taps.
  lc chain (row g, col g+rot+1): 8 taps of k[dy,dx]*ones (center k=0).
    -> L state kept in "rotated by rot+1" layout.
  T1 = 1 + 0.5*L is rotated back to rot via identity matmuls (pT chain).

Per-chunk partition rotation: chunk state (F/L/E/flat-y/psum) for chunks
{2,3} is stored rotated +2 partition groups vs chunks {0,1}.  Elementwise
ops are layout-agnostic; conv-input y planes stay natural.  Consecutive
chunks in processing order (1,2,0,3) then use DISJOINT 32x32 PE tilesets
(col rotations 0/1 vs 2/3), so two chunks' matmul chains stream through
the PE array concurrently (up to 16 active tiles instead of 8).
Order (1,2,0,3) also lets step t+1's first chunk start before step t's
last chunk finishes its elementwise tail (keeps PE warm, HAM at 8/8).
"""

import numpy as np

import concourse.bass as bass
import concourse.mybir as mybir
import concourse.tile as tile
from concourse.bass_utils import run_bass_kernel_spmd


F32 = mybir.dt.float32
BF16 = mybir.dt.bfloat16
ALU = mybir.AluOpType
ACTF = mybir.ActivationFunctionType

ALPHA_F, ALPHA_L, ALPHA_E, V_E = 0.1, 1.0, 1.0, 10.0
DF = float(np.exp(-ALPHA_F))
DL = float(np.exp(-ALPHA_L))
DE = float(np.exp(-ALPHA_E))

B, T, C, H, W = 8, 10, 32, 128, 128
G = 4  # partition groups / row bands
KVALS = [[0.5, 1.0, 0.5], [1.0, 0.0, 1.0], [0.5, 1.0, 0.5]]

CHUNK_ORDER = [0, 1, 3, 2]
ROT = {0: 0, 1: 2, 3: 0, 2: 2}


def build_nc(t_steps=T, h=H, w=W):
    br = h // G          # rows per band
    pw = w + 2           # padded width
    ph = br + 2          # padded rows per band
    n_chunks = br // 8   # 8 image rows (2 psum banks) per chunk
    assert br % 8 == 0 and w == 128

    nc = bass.Bass()
    x_ext = nc.declare_dram_parameter("x", [t_steps, C, h, w], F32, isOutput=False)
    w_ext = nc.declare_dram_parameter("conv_w", [C, C, 3, 3], F32, isOutput=False)
    b_ext = nc.declare_dram_parameter("conv_b", [C], F32, isOutput=False)
    y_ext = nc.declare_dram_parameter("y", [t_steps, C, h, w], F32, isOutput=True)

    from contextlib import ExitStack
    with tile.TileContext(nc) as tc, ExitStack() as ctx:
        _build(ctx, tc, nc, x_ext, w_ext, b_ext, y_ext, t_steps, h, w, br, ph, pw, n_chunks)
    return nc


def _split_matmul_waits(nc):
    """walrus's S3_LW (matmul weight-load) struct has a single sync-wait
    slot; Tile sometimes attaches 2+. Move all-but-one wait onto an
    inserted PE EventSemaphore right before the matmul."""
    import copy as _copy

    split_types = {
        "InstMatmult", "InstTensorTensor", "InstTensorScalarPtr",
        "InstActivation", "InstTensorCopy", "InstStreamTranspose",
        "InstMemset", "InstTensorScalarAffineSelect", "InstTensorReduce",
        "InstDMACopy", "InstTensorLoad", "InstTensorSave", "InstDrain", "InstNoOp",
    }
    fn = nc.m.functions[0]
    new_blocks = []
    for bb in fn.blocks:
        out = []
        changed = False
        for ins in bb.instructions:
            si = ins.sync_info
            if (type(ins).__name__ in split_types and si is not None
                    and si.on_wait and len(si.on_wait) > 1):
                waits = list(si.on_wait)
                for i in range(0, len(waits), 2):  # <=2 waits per EventSemaphore
                    out.append(mybir.InstEventSemaphore(
                        name=nc.get_next_instruction_name(),
                        engine=ins.engine,
                        ins=[], outs=[],
                        sync_info=mybir.SyncInfo(
                            on_wait=waits[i:i + 2], on_update=[]),
                    ))
                ins.sync_info = mybir.SyncInfo(
                    on_wait=[], on_update=list(si.on_update or []))
                changed = True
            out.append(ins)
        if changed:
            new_blocks.append(_copy.replace(bb, instructions=out))
        else:
            new_blocks.append(bb)
    new_fn = _copy.replace(fn, blocks=[])
    new_fn.set_allocations_from_list(fn.allocations)
    new_fn.blocks.extend(new_blocks)
    nc.m = _copy.replace(nc.m, functions=[])
    nc.m.functions.append(new_fn)


def _build(ctx, tc, nc, x_ext, w_ext, b_ext, y_ext, t_steps, h, w, br, ph, pw, n_chunks):
    singles = ctx.enter_context(tc.tile_pool(name="singles", bufs=1))
    xbpool = ctx.enter_context(tc.tile_pool(name="xbpool", bufs=1))
    xpool = ctx.enter_context(tc.tile_pool(name="xpool", bufs=2))
    tmp_pool = ctx.enter_context(tc.tile_pool(name="tmps", bufs=2))
    psA_pool = ctx.enter_context(tc.tile_pool(name="psA", bufs=2, space="PSUM"))
    psB_pool = ctx.enter_context(tc.tile_pool(name="psB", bufs=2, space="PSUM"))

    # ---- persistent state ----
    Yf0 = singles.tile([128, br * w], F32, tag="Yf0")   # flat fp32 y (ping)
    Yf1 = singles.tile([128, br * w], F32, tag="Yf1")   # flat fp32 y (pong)
    Yh0 = singles.tile([128, ph * pw], BF16, tag="Yh0")  # padded bf16 hi
    Yl0 = singles.tile([128, ph * pw], BF16, tag="Yl0")  # padded bf16 lo
    Yh1 = singles.tile([128, ph * pw], BF16, tag="Yh1")
    Yl1 = singles.tile([128, ph * pw], BF16, tag="Yl1")
    F = singles.tile([128, br * w], F32, tag="F")
    L = singles.tile([128, br * w], F32, tag="L")   # rotated: group (g+rot+1)%4 = band g
    E = singles.tile([128, br * w], F32, tag="E")   # ehat = e / V_E

    # ---- constants ----
    Wcf = singles.tile([128, 9 * 32], F32, tag="Wcf")    # fp32 staging, replicated
    Wch = singles.tile([128, 9 * 32], BF16, tag="Wch")   # bf16 hi
    Wcl = singles.tile([128, 9 * 32], BF16, tag="Wcl")   # bf16 lo
    Wstage = singles.tile([32, 9 * 32], F32, tag="Wstage")
    Bcol = singles.tile([128, 1], F32, tag="Bcol")       # per-partition bias
    IDT = singles.tile([128, 32], BF16, tag="IDT")
    K05 = singles.tile([128, 32], BF16, tag="K05")
    K10 = singles.tile([128, 32], BF16, tag="K10")

    for yy in (Yh0, Yl0, Yh1, Yl1):
        nc.vector.memset(yy, 0.0)
    nc.vector.memset(Yf0, 0.0)
    nc.vector.memset(Yf1, 0.0)
    nc.vector.memset(F, 0.0)
    nc.vector.memset(L, 0.0)
    # e0 = V_E/ALPHA_E, ehat0 = e0/V_E = 1/ALPHA_E
    nc.vector.memset(E, 1.0 / ALPHA_E)
    nc.vector.memset(K05, 0.5)
    nc.vector.memset(K10, 1.0)

    # conv weights: stage as [o, (i ky kx)] contiguous, transpose per tap to [i, o]
    nc.sync.dma_start(out=Wstage, in_=w_ext[:].rearrange("o i ky kx -> o (i ky kx)"))
    wst = Wstage.rearrange("p (i t) -> p i t", t=9)
    for tap in range(9):
        nc.vector.transpose(out=Wcf[0:32, 32 * tap:32 * tap + 32], in_=wst[:, :, tap])
    # identity: gpsimd can't touch f32r, so build in an f32 staging tile
    # and cast-copy into IDT on DVE (exact for 0/1 values)
    IDTs = singles.tile([32, 32], F32, tag="IDTs")
    nc.gpsimd.memset(IDTs, 0.0)
    nc.gpsimd.affine_select(
        out=IDTs, in_=IDTs,
        compare_op=ALU.not_equal, fill=1.0, base=0,
        pattern=[[-1, 32]], channel_multiplier=1)
    nc.vector.tensor_copy(out=IDT[0:32, :], in_=IDTs)
    # per-partition bias column: partition 32g+c -> conv_b[c]
    b_ap = b_ext[:]
    b_pcast = bass.AP(tensor=b_ap.tensor, offset=b_ap.offset,
                      ap=[[0, G]] + list(b_ap.ap) + [[0, 1]])
    nc.sync.dma_start(out=Bcol, in_=b_pcast)
    # replicate fp32 weights to groups 1..3, then hi/lo bf16 split
    for g in range(1, G):
        nc.vector.tensor_copy(out=Wcf[32 * g:32 * g + 32, :], in_=Wcf[0:32, :])
        nc.vector.tensor_copy(out=IDT[32 * g:32 * g + 32, :], in_=IDT[0:32, :])
    nc.vector.tensor_copy(out=Wch, in_=Wcf)
    nc.vector.scalar_tensor_tensor(out=Wcl, in0=Wcf, scalar=1.0, in1=Wch,
                                   op0=ALU.mult, op1=ALU.subtract)

    taps = [(ky, kx) for ky in range(3) for kx in range(3)]
    lc_taps = [(ky, kx) for (ky, kx) in taps if KVALS[ky][kx] != 0.0]

    def emit_stage2(c):
        """Post-PSUM tail of a chunk: pT rotate-back, u/e/T2, sigmoid,
        y plane writes, y DMA, halo copies.  Emitted AFTER the next
        chunk's conv matmuls so the pT matmuls never head-of-line-block
        the PE queue (their DVE-chain deps resolve during the next
        chunk's streaming)."""
        t, ch, rot, sl = c["t"], c["ch"], c["rot"], c["sl"]
        T1h, T1l = c["T1h"], c["T1l"]
        yf_in, yf_out = c["yf_in"], c["yf_out"]
        yhov, ylov = c["yhov"], c["ylov"]
        pT = psB_pool.tile([128, 1024], F32, tag="pBT")
        for lb in range(2):
            co = 512 * lb
            for hi, tt in enumerate((T1h, T1l)):
                for g in range(G):
                    gr1 = (g + rot + 1) % G
                    gr0 = (g + rot) % G
                    nc.tensor.matmul(
                        out=pT[32 * gr0:32 * gr0 + 32, co:co + 512],
                        lhsT=IDT[32 * gr1:32 * gr1 + 32, 0:32],
                        rhs=tt[32 * gr1:32 * gr1 + 32, co:co + 512],
                        start=(hi == 0), stop=(hi == 1),
                        skip_group_check=True, tile_position=(32 * gr1, 32 * gr0),
                    )
        U = tmp_pool.tile([128, 1024], F32, tag="U")
        nc.vector.tensor_tensor(out=U, in0=F[:, sl], in1=pT[:, :], op=ALU.mult)
        # ehat = DE*ehat + y_old (flat fp32 y)
        nc.vector.scalar_tensor_tensor(
            out=E[:, sl], in0=E[:, sl], scalar=DE, in1=yf_in[:, sl],
            op0=ALU.mult, op1=ALU.add)
        nc.vector.scalar_tensor_tensor(
            out=U, in0=E[:, sl], scalar=-V_E, in1=U,
            op0=ALU.mult, op1=ALU.add)
        nc.scalar.activation(out=yf_out[:, sl], in_=U, func=ACTF.Sigmoid)
        # bf16 hi/lo split of new y into padded planes, un-rotating
        # (band g's chunk data lives at partition group (g+rot)%4).
        # 2-input ALU ops need equal SBUF base partitions, so for rot!=0
        # split hi/lo in rotated layout first, then un-rotate both with
        # 1-input copies (scalar/vector handle cross-partition strides).
        if rot == 0:
            src = yf_out[:, sl].rearrange("p (r c) -> p r c", c=w)
            yho_int = yhov[:, 8 * ch + 1:8 * ch + 9, 1:1 + w]
            ylo_int = ylov[:, 8 * ch + 1:8 * ch + 9, 1:1 + w]
            nc.scalar.copy(out=yho_int, in_=src)
            nc.gpsimd.tensor_tensor(out=ylo_int, in0=src, in1=yho_int,
                                    op=ALU.subtract)
        else:
            Yhr = tmp_pool.tile([128, 1024], BF16, tag="Yhr")
            Ylr = tmp_pool.tile([128, 1024], BF16, tag="Ylr")
            nc.scalar.copy(out=Yhr, in_=yf_out[:, sl])
            nc.gpsimd.tensor_tensor(out=Ylr, in0=yf_out[:, sl], in1=Yhr,
                                    op=ALU.subtract)
            for (p0, p1) in ((0, 64), (64, 128)):
                s0 = (p0 + 32 * rot) % 128
                npt = p1 - p0
                nc.scalar.copy(
                    out=yhov[p0:p1, 8 * ch + 1:8 * ch + 9, 1:1 + w],
                    in_=Yhr[s0:s0 + npt, :].rearrange("p (r c) -> p r c", c=w))
                nc.vector.tensor_copy(
                    out=ylov[p0:p1, 8 * ch + 1:8 * ch + 9, 1:1 + w],
                    in_=Ylr[s0:s0 + npt, :].rearrange("p (r c) -> p r c", c=w))
        y_dst = y_ext[t].rearrange("c (g r) w -> g c r w", g=G)
        for g in range(G):
            g2 = (g + rot) % G
            nc.sync.dma_start(
                out=y_dst[g, :, 8 * ch:8 * ch + 8, :],
                in_=yf_out[32 * g2:32 * g2 + 32, sl])
        # halo rows for next step's convs, as soon as the producing
        # chunk's plane rows are written:
        #   chunk 0 produced band-first rows -> bottom halos (row br+1)
        #   chunk 3 produced band-last rows  -> top halos (row 0)
        if t + 1 < t_steps and ch == 0:
            for yv in (yhov, ylov):
                for g in range(G - 1):
                    nc.vector.tensor_copy(out=yv[32 * g:32 * g + 32, br + 1, :],
                                          in_=yv[32 * (g + 1):32 * (g + 2), 1, :])
        if t + 1 < t_steps and ch == 3:
            for yv in (yhov, ylov):
                for g in range(1, G):
                    nc.vector.tensor_copy(out=yv[32 * g:32 * g + 32, 0, :],
                                          in_=yv[32 * (g - 1):32 * g, br, :])

    prev = None
    for t in range(t_steps):
        yf_in = Yf0 if t % 2 == 0 else Yf1
        yf_out = Yf1 if t % 2 == 0 else Yf0
        yh_in, yl_in = (Yh0, Yl0) if t % 2 == 0 else (Yh1, Yl1)
        yh_out, yl_out = (Yh1, Yl1) if t % 2 == 0 else (Yh0, Yl0)
        yhv = yh_in.rearrange("p (r c) -> p r c", c=pw)
        ylv = yl_in.rearrange("p (r c) -> p r c", c=pw)
        yhov = yh_out.rearrange("p (r c) -> p r c", c=pw)
        ylov = yl_out.rearrange("p (r c) -> p r c", c=pw)

        xb = xbpool.tile([128, br * w], F32, tag="xb")
        nc.sync.dma_start(out=xb, in_=x_ext[t].rearrange("c (g r) w -> g c r w", g=G))
        xh = xpool.tile([128, br * w], BF16, tag="xh")
        xl = xpool.tile([128, br * w], BF16, tag="xl")
        # xh + xl == (x + bias) to ~2^-17: bias folded into the split
        nc.scalar.activation(out=xh, in_=xb, func=ACTF.Identity,
                             bias=Bcol[:, 0:1], scale=1.0)
        nc.vector.scalar_tensor_tensor(out=xl, in0=xb, scalar=Bcol[:, 0:1],
                                       op0=ALU.add, op1=ALU.subtract, in1=xh)

        for ch in CHUNK_ORDER:
            rot = ROT[ch]
            sl = slice(1024 * ch, 1024 * ch + 1024)
            pA = psA_pool.tile([128, 1024], F32, tag="pA")
            pB = psB_pool.tile([128, 1024], F32, tag="pBT")
            for lb in range(2):            # local bank
                gb = 2 * ch + lb           # global 4-row block index
                co = 512 * lb
                # cf: 27 split-conv taps + (x+bias) hi/lo (col-rotated tiles)
                for ti, (ky, kx) in enumerate(taps):
                    for (wt, yv) in ((Wch, yhv), (Wch, ylv), (Wcl, yhv)):
                        for g in range(G):
                            g2 = (g + rot) % G
                            nc.tensor.matmul(
                                out=pA[32 * g2:32 * g2 + 32, co:co + 512],
                                lhsT=wt[32 * g:32 * g + 32, 32 * ti:32 * ti + 32],
                                rhs=yv[32 * g:32 * g + 32, 4 * gb + ky:4 * gb + ky + 4,
                                       kx:kx + 128],
                                start=(ti == 0 and wt is Wch and yv is yhv),
                                stop=False, skip_group_check=True,
                                tile_position=(32 * g, 32 * g2),
                            )
                for xi, xt in enumerate((xh, xl)):
                    for g in range(G):
                        g2 = (g + rot) % G
                        nc.tensor.matmul(
                            out=pA[32 * g2:32 * g2 + 32, co:co + 512],
                            lhsT=IDT[32 * g:32 * g + 32, 0:32],
                            rhs=xt[32 * g:32 * g + 32, 512 * gb:512 * gb + 512],
                            start=False, stop=(xi == 1), skip_group_check=True,
                            tile_position=(32 * g, 32 * g2),
                        )
                # lc: 8 taps x (yh, yl), output rotated by rot+1
                nlc = len(lc_taps)
                for ti, (ky, kx) in enumerate(lc_taps):
                    kt = K05 if KVALS[ky][kx] == 0.5 else K10
                    for yi, yv in enumerate((yhv, ylv)):
                        for g in range(G):
                            g3 = (g + rot + 1) % G
                            nc.tensor.matmul(
                                out=pB[32 * g3:32 * g3 + 32, co:co + 512],
                                lhsT=kt[32 * g:32 * g + 32, 0:32],
                                rhs=yv[32 * g:32 * g + 32, 4 * gb + ky:4 * gb + ky + 4,
                                       kx:kx + 128],
                                start=(ti == 0 and yi == 0),
                                stop=(ti == nlc - 1 and yi == 1),
                                skip_group_check=True, tile_position=(32 * g, 32 * g3),
                            )

            # stage1: state updates off the fresh PSUM banks
            nc.vector.scalar_tensor_tensor(
                out=F[:, sl], in0=F[:, sl], scalar=DF, in1=pA[:, :],
                op0=ALU.mult, op1=ALU.add)
            nc.vector.scalar_tensor_tensor(
                out=L[:, sl], in0=L[:, sl], scalar=DL, in1=pB[:, :],
                op0=ALU.mult, op1=ALU.add)
            T1f = tmp_pool.tile([128, 1024], F32, tag="T1f")
            T1h = tmp_pool.tile([128, 1024], BF16, tag="T1h")
            T1l = tmp_pool.tile([128, 1024], BF16, tag="T1l")
            nc.vector.tensor_scalar(T1f, L[:, sl], 0.5, 1.0, ALU.mult, ALU.add)
            nc.scalar.copy(out=T1h, in_=T1f)
            nc.gpsimd.tensor_tensor(out=T1l, in0=T1f, in1=T1h, op=ALU.subtract)
            if prev is not None:
                emit_stage2(prev)
            prev = {"t": t, "ch": ch, "rot": rot, "sl": sl, "T1h": T1h,
                    "T1l": T1l, "yf_in": yf_in, "yf_out": yf_out,
                    "yhov": yhov, "ylov": ylov}
    emit_stage2(prev)


_NC_CACHE = {}


def _get_nc(t_steps, h, w):
    key = (t_steps, h, w)
    if key not in _NC_CACHE:
        nc = build_nc(t_steps, h, w)
        _split_matmul_waits(nc)   # HW compile path only; CoreSim can't run these
        _NC_CACHE[key] = nc
    return _NC_CACHE[key]


def kernel(x, conv_w, conv_b):
    x = np.ascontiguousarray(np.asarray(x), dtype=np.float32)
    conv_w = np.ascontiguousarray(np.asarray(conv_w), dtype=np.float32)
    conv_b = np.ascontiguousarray(np.asarray(conv_b), dtype=np.float32)
    b, t_steps, c, h, w = x.shape
    nc = _get_nc(t_steps, h, w)
    in_maps = [
        {"x": x[i], "conv_w": conv_w, "conv_b": conv_b} for i in range(b)
    ]
    res = run_bass_kernel_spmd(nc, in_maps, core_ids=list(range(b)))
    return np.stack([res.results[i]["y"] for i in range(b)], axis=0)


if __name__ == "__main__":
    nc = build_nc()
    print("built ok")
